# revision 21
# baseline (speedup 1.0000x reference)
"""KNN (retrieval_knn) Trainium2 Bass kernel, v3.

Problem: xyz (8, 16384, 3) f32, centers (8, 1024, 3) f32 ->
top-16 nearest points per center, indices, shape (8, 16, 1024) int32.

Data-parallel over batch B=8 across 8 NeuronCores (identical SPMD program).

v3 dispatch: per-call wall clock on this axon-tunneled setup is dominated
by ~80-95ms tunnel round trips, not device time (CoreSim ~0.34ms/core).
kernel() AOT-compiles the 8-core shard_map ONCE (fast_dispatch_compile)
and never blocks between dispatch and output fetch, so upload + execute +
fetch pipeline into a single round trip (~88ms/call vs ~300ms for stock
per-call run_bass_kernel_spmd). Output indices travel as u16 (N=16384
fits) to halve the return payload; host casts back to int32.

Per core:
  - Distance via -d2[k, n] = 2c.x - x_sq - c_sq computed as ONE bf16 matmul
    with a 30-row contraction: every fp32 operand is split into three bf16
    pieces (a ~= a1+a2+a3) and the significant cross products are separate
    contraction rows. HW-validated precision: ~5e-7 mean abs err (fp32-mode
    matmul: ~3e-7) at 1 cycle/col instead of fp32's 4.
  - Operand prep runs in a dense [128, 384] layout (cheap: engine cost is
    per-partition free size), then one DRAM round-trip rearranges it into
    the wide [30, 16384] moving layout.
  - ACT evicts pairs of 1024-wide PSUM tiles (2 banks each) into 2048-wide
    SBUF scan buffers; DVE max8 + max_index over those (cheaper SBUF
    access, half the scan instructions vs PSUM-direct) -> 64 candidates
    per row; merge via max8/match_replace/max8 + max_index; recover global
    indices with an exact one-hot gather; PE-transpose to [16, 128]; emit
    int32. Top-8-per-2048-tile capacity is probabilistic (top-16 members
    spread over 8 tiles): expected ~2 rows per full problem lose a rank-
    ~16 member (HW-measured: 19 total mismatches, rel err 0.008 vs the
    0.02 gate).

CoreSim: 342.6 us (baseline kernel: 630.7 us). The one-hot gather's
casts/adds/multiply and the output copy run on gpsimd/ACT (fp32 TT
mult/add + casts are Pool-legal; is_equal and free-dim reduce are not). DVE-bound (two full
max8/max_index scans = 87% of DVE busy); ACT eviction runs within ~3% of
DVE busy, so both are near-saturated. Next step would need a cheaper
first pass (bf16 2x tournament + per-partition window gather; selection
was HW-validated but all per-partition gather paths are blocked -- see
memory trn2-knn-kernel-env-facts).
"""

import numpy as np

B = 8
N = 16384
K = 1024
KNN = 16
TILE = 1024               # PSUM tile width (2 banks)
NT = N // TILE            # 16 PSUM tiles per row
SCAN = 2048               # DVE scan-tile width (SBUF, ACT-evicted)
NBLK = K // 128           # 8 center blocks
NCAND = (N // SCAN) * 8   # 64 candidates per row
NEG = -3.0e38

# 30-row split contraction layout.
# x-side rows:              w-side rows:
#  0- 8: sqx_c split l      -1
#  9-11: 1.0                -csq split l
# 12-29: per coord c (6 rows): x-side [x1,x1,x1,x2,x2,x3]
#                             w-side [v1,v2,v3,v1,v2,v1], v = 2c
NROW = 30

_CACHE = {}


def _build():
    import concourse.bass as bass
    import concourse.mybir as mybir
    import concourse.tile as tile
    from concourse import bacc
    from concourse.masks import make_identity

    fp32 = mybir.dt.float32
    bf16 = mybir.dt.bfloat16
    u16 = mybir.dt.uint16
    i32 = mybir.dt.int32

    nc = bacc.Bacc("TRN2", target_bir_lowering=False, debug=False)

    xyz_d = nc.dram_tensor("xyz", [N, 3], fp32, kind="ExternalInput")
    cen_d = nc.dram_tensor("centers", [K, 3], fp32, kind="ExternalInput")
    # u16 output (indices < 16384): halves the per-call tunnel fetch
    # payload vs int32; host casts back to int32 after the fetch.
    out_d = nc.dram_tensor("out_idx", [KNN, K], u16, kind="ExternalOutput")
    # DRAM bounce buffers for the dense->wide rearrangement
    xb_d = nc.dram_tensor("xb_bounce", [NROW * N], bf16)
    wb_d = nc.dram_tensor("wb_bounce", [NROW * K], bf16)

    PTS = N // 128            # 128 points per partition in dense layout
    CTR = K // 128            # 8 centers per partition in dense layout

    with tile.TileContext(nc) as tc:
        with (
            tc.tile_pool(name="wide", bufs=1) as wide,
            tc.tile_pool(name="prep", bufs=1) as prep,
            tc.tile_pool(name="cand", bufs=3) as cand,
            tc.tile_pool(name="small", bufs=1) as small,
            tc.tile_pool(name="ps_mm", bufs=3, space="PSUM") as ps_mm,
            tc.tile_pool(name="ps_tr", bufs=2, space="PSUM") as ps_tr,
        ):
            # ---------------- constants ----------------
            ramp = small.tile([128, NCAND], fp32, tag="ramp")
            nc.gpsimd.iota(ramp[:], [[1, NCAND]], channel_multiplier=0,
                           allow_small_or_imprecise_dtypes=True)
            offs = small.tile([128, NCAND], fp32, tag="offs")
            nc.gpsimd.iota(offs[:], [[SCAN, N // SCAN], [0, 8]],
                           channel_multiplier=0,
                           allow_small_or_imprecise_dtypes=True)
            ident = small.tile([128, 128], fp32, tag="ident")
            make_identity(nc, ident[:])

            # ---------------- dense x-side prep ----------------
            # praw[p, c*128 + j] = coord c of point 128p + j
            praw = prep.tile([128, 3 * PTS], fp32, tag="praw")
            for c in range(3):
                nc.sync.dma_start(praw[:, c * PTS:(c + 1) * PTS],
                                  xyz_d[:, c:c + 1])
            sqx = prep.tile([128, 3 * PTS], fp32, tag="sqx")
            nc.scalar.square(sqx[:], praw[:])

            # srcall[p, r*128 + j] = x-side row r of point 128p + j.
            # The split casts write DIRECTLY into srcall's column ranges
            # (with stride-0 input broadcast for the duplicated x rows),
            # removing six serial assembly copies from the startup chain.
            srcall = prep.tile([128, NROW * PTS], bf16, tag="srcall")
            # x-side views: row 12+6c+r for coord c holds [x1,x1,x1,x2,x2,x3]
            xv = srcall[:, 12 * PTS:30 * PTS].rearrange(
                "p (c r j) -> p c r j", c=3, r=6)

            # 3-way bf16 splits of praw (casts write srcall dup ranges)
            xr1 = prep.tile([128, 3 * PTS], fp32, tag="xr1")
            xr2 = prep.tile([128, 3 * PTS], fp32, tag="xr2")
            prv = praw[:].rearrange("p (c j) -> p c j", c=3)
            nc.gpsimd.tensor_copy(
                xv[:, :, 0:3, :],
                prv[:, :, None, :].broadcast_to([128, 3, 3, PTS]))
            nc.vector.tensor_tensor(
                xr1[:].rearrange("p (c j) -> p c j", c=3),
                prv, xv[:, :, 0, :], mybir.AluOpType.subtract)
            x1v = xr1[:].rearrange("p (c j) -> p c j", c=3)
            nc.gpsimd.tensor_copy(
                xv[:, :, 3:5, :],
                x1v[:, :, None, :].broadcast_to([128, 3, 2, PTS]))
            nc.vector.tensor_tensor(
                xr2[:].rearrange("p (c j) -> p c j", c=3),
                x1v, xv[:, :, 3, :], mybir.AluOpType.subtract)
            nc.gpsimd.tensor_copy(xv[:, :, 5, :],
                                  xr2[:].rearrange("p (c j) -> p c j", c=3))

            # 3-way bf16 splits of sqx -> srcall rows 0-8 directly
            qr1 = prep.tile([128, 3 * PTS], fp32, tag="qr1")
            qr2 = prep.tile([128, 3 * PTS], fp32, tag="qr2")
            nc.gpsimd.tensor_copy(srcall[:, 0 * PTS:3 * PTS], sqx[:])
            nc.vector.tensor_tensor(qr1[:], sqx[:], srcall[:, 0 * PTS:3 * PTS],
                                    mybir.AluOpType.subtract)
            nc.gpsimd.tensor_copy(srcall[:, 3 * PTS:6 * PTS], qr1[:])
            nc.vector.tensor_tensor(qr2[:], qr1[:], srcall[:, 3 * PTS:6 * PTS],
                                    mybir.AluOpType.subtract)
            nc.gpsimd.tensor_copy(srcall[:, 6 * PTS:9 * PTS], qr2[:])
            # rows 9-11: ones
            nc.gpsimd.memset(srcall[:, 9 * PTS:12 * PTS], 1.0)

            # bounce: srcall -> DRAM in (row, point) order -> wide xb
            # DRAM flat index = r*N + 128*p + j ; both APs walk (p, r, j).
            # Chunked over partition ranges so the chunked readback below
            # unblocks as early as possible.
            nc.sync.dma_start(
                xb_d.ap().rearrange("(r p j) -> p r j", r=NROW, p=128, j=PTS),
                srcall[:].rearrange("p (r j) -> p r j", r=NROW),
            )

            # ---------------- dense w-side prep ----------------
            craw = prep.tile([128, 3 * CTR], fp32, tag="craw")
            for c in range(3):
                nc.sync.dma_start(craw[:, c * CTR:(c + 1) * CTR],
                                  cen_d[:, c:c + 1])
            v2c = prep.tile([128, 3 * CTR], fp32, tag="v2c")
            nc.scalar.mul(v2c[:], craw[:], 2.0)
            sqc = prep.tile([128, 3 * CTR], fp32, tag="sqc")
            nc.scalar.square(sqc[:], craw[:])
            ncsq = prep.tile([128, CTR], fp32, tag="ncsq")
            nc.vector.tensor_tensor(ncsq[:], sqc[:, 0:CTR], sqc[:, CTR:2 * CTR],
                                    mybir.AluOpType.add)
            nc.vector.tensor_tensor(ncsq[:], ncsq[:], sqc[:, 2 * CTR:3 * CTR],
                                    mybir.AluOpType.add)
            nc.scalar.mul(ncsq[:], ncsq[:], -1.0)

            vs1 = prep.tile([128, 3 * CTR], bf16, tag="vs1")
            vr1 = prep.tile([128, 3 * CTR], fp32, tag="vr1")
            vs2 = prep.tile([128, 3 * CTR], bf16, tag="vs2")
            vr2 = prep.tile([128, 3 * CTR], fp32, tag="vr2")
            vs3 = prep.tile([128, 3 * CTR], bf16, tag="vs3")
            nc.vector.tensor_copy(vs1[:], v2c[:])
            nc.vector.tensor_tensor(vr1[:], v2c[:], vs1[:],
                                    mybir.AluOpType.subtract)
            nc.vector.tensor_copy(vs2[:], vr1[:])
            nc.vector.tensor_tensor(vr2[:], vr1[:], vs2[:],
                                    mybir.AluOpType.subtract)
            nc.vector.tensor_copy(vs3[:], vr2[:])

            cs1 = prep.tile([128, CTR], bf16, tag="cs1")
            cr1 = prep.tile([128, CTR], fp32, tag="cr1")
            cs2 = prep.tile([128, CTR], bf16, tag="cs2")
            cr2 = prep.tile([128, CTR], fp32, tag="cr2")
            cs3 = prep.tile([128, CTR], bf16, tag="cs3")
            nc.vector.tensor_copy(cs1[:], ncsq[:])
            nc.vector.tensor_tensor(cr1[:], ncsq[:], cs1[:],
                                    mybir.AluOpType.subtract)
            nc.vector.tensor_copy(cs2[:], cr1[:])
            nc.vector.tensor_tensor(cr2[:], cr1[:], cs2[:],
                                    mybir.AluOpType.subtract)
            nc.vector.tensor_copy(cs3[:], cr2[:])

            wsrc = prep.tile([128, NROW * CTR], bf16, tag="wsrc")
            # rows 0-8: -1
            nc.gpsimd.memset(wsrc[:, 0:9 * CTR], -1.0)
            # rows 9-11: -csq splits
            nc.vector.tensor_copy(wsrc[:, 9 * CTR:10 * CTR], cs1[:])
            nc.vector.tensor_copy(wsrc[:, 10 * CTR:11 * CTR], cs2[:])
            nc.vector.tensor_copy(wsrc[:, 11 * CTR:12 * CTR], cs3[:])
            # rows 12-29: per coord c: [v1, v2, v3, v1, v2, v1]
            for c in range(3):
                base = (12 + 6 * c) * CTR
                sl1 = vs1[:, c * CTR:(c + 1) * CTR]
                sl2 = vs2[:, c * CTR:(c + 1) * CTR]
                sl3 = vs3[:, c * CTR:(c + 1) * CTR]
                nc.vector.tensor_copy(wsrc[:, base + 0 * CTR:base + 1 * CTR], sl1)
                nc.vector.tensor_copy(wsrc[:, base + 1 * CTR:base + 2 * CTR], sl2)
                nc.vector.tensor_copy(wsrc[:, base + 2 * CTR:base + 3 * CTR], sl3)
                nc.vector.tensor_copy(wsrc[:, base + 3 * CTR:base + 4 * CTR], sl1)
                nc.vector.tensor_copy(wsrc[:, base + 4 * CTR:base + 5 * CTR], sl2)
                nc.vector.tensor_copy(wsrc[:, base + 5 * CTR:base + 6 * CTR], sl1)

            nc.sync.dma_start(
                wb_d.ap().rearrange("(r p j) -> p r j", r=NROW, p=128, j=CTR),
                wsrc[:].rearrange("p (r j) -> p r j", r=NROW),
            )

            # wide operands back from DRAM; xb readback in column chunks so
            # block 0's first matmuls only wait for chunk 0
            xb = wide.tile([NROW, N], bf16, tag="xb")
            xbv = xb_d.ap().rearrange("(r n) -> r n", r=NROW)
            XCH = N // 4
            for ch in range(4):
                nc.sync.dma_start(xb[:, ch * XCH:(ch + 1) * XCH],
                                  xbv[:, ch * XCH:(ch + 1) * XCH])
            wb = wide.tile([NROW, K], bf16, tag="wb")
            nc.sync.dma_start(wb[:], wb_d.ap().rearrange("(r k) -> r k", r=NROW))

            # ---------------- output accumulator ----------------
            out_sb = small.tile([KNN, K], u16, tag="out")

            # ---------------- main loop over center blocks ----------------
            # Software-pipelined: block b's merge/gather tail is emitted
            # after block b+1's scans so DVE's in-order stream never stalls
            # on the tail's Pool/ACT round trips.
            def scans(blk):
                v = cand.tile([128, NCAND], fp32, tag="v")
                p = cand.tile([128, NCAND], u16, tag="p")
                for t in range(N // SCAN):
                    # ACT evicts two PSUM tiles into one wide SBUF scan
                    # buffer: DVE's max8/max_index then pay the cheaper SBUF
                    # access penalty, half as many scan instructions, and
                    # the candidate set shrinks to 64.
                    rb = cand.tile([128, SCAN], fp32, tag="rb")
                    for g in range(SCAN // TILE):
                        ps = ps_mm.tile([128, TILE], fp32, tag="mm")
                        base = t * SCAN + g * TILE
                        for h in range(2):
                            nc.tensor.matmul(
                                ps[:, h * 512:(h + 1) * 512],
                                wb[:, blk * 128:(blk + 1) * 128],
                                xb[:, base + h * 512:base + (h + 1) * 512],
                                start=True, stop=True,
                            )
                        nc.scalar.copy(rb[:, g * TILE:(g + 1) * TILE], ps[:])
                    nc.vector.max(out=v[:, t * 8:t * 8 + 8], in_=rb[:])
                    nc.vector.max_index(p[:, t * 8:t * 8 + 8],
                                        v[:, t * 8:t * 8 + 8], rb[:])
                return v, p

            def tail(blk, v, p):

                # merge: top-16 of the 128 candidates
                w1 = cand.tile([128, 8], fp32, tag="w1")
                w2 = cand.tile([128, 8], fp32, tag="w2")
                q = cand.tile([128, KNN], u16, tag="q")
                v2 = cand.tile([128, NCAND], fp32, tag="v2")
                nc.vector.max(out=w1[:], in_=v[:])
                nc.vector.max_index(q[:, 0:8], w1[:], v[:])
                nc.vector.match_replace(out=v2[:], in_to_replace=w1[:],
                                        in_values=v[:], imm_value=NEG)
                nc.vector.max(out=w2[:], in_=v2[:])
                nc.vector.max_index(q[:, 8:16], w2[:], v2[:])

                # exact one-hot gather: GI[j] = sum_c (q[j]==c) * (p[c]+offs[c])
                # Casts/adds/the one-hot multiply run on the otherwise-idle
                # gpsimd engine (fp32 TT mult/add + casts are Pool-legal;
                # is_equal and free-dim reduce are DVE-only).
                pf = cand.tile([128, NCAND], fp32, tag="pf")
                nc.gpsimd.tensor_copy(pf[:], p[:])
                pg = cand.tile([128, NCAND], fp32, tag="pg")
                nc.gpsimd.tensor_tensor(pg[:], pf[:], offs[:],
                                        mybir.AluOpType.add)
                qf = cand.tile([128, KNN], fp32, tag="qf")
                nc.gpsimd.tensor_copy(qf[:], q[:])
                eq = cand.tile([128, KNN, NCAND], fp32, tag="eq")
                nc.vector.tensor_tensor(
                    eq[:],
                    qf[:, :, None].broadcast_to([128, KNN, NCAND]),
                    ramp[:, None, :].broadcast_to([128, KNN, NCAND]),
                    mybir.AluOpType.is_equal,
                )
                eqw = cand.tile([128, KNN, NCAND], fp32, tag="eqw")
                nc.gpsimd.tensor_tensor(
                    eqw[:], eq[:],
                    pg[:, None, :].broadcast_to([128, KNN, NCAND]),
                    mybir.AluOpType.mult,
                )
                gi = cand.tile([128, KNN], fp32, tag="gi")
                nc.vector.tensor_reduce(gi[:], eqw[:], axis=mybir.AxisListType.X,
                                        op=mybir.AluOpType.add)

                # transpose [128, 16] -> [16, 128] and emit int32
                pst = ps_tr.tile([KNN, 128], fp32, tag="tr")
                nc.tensor.transpose(pst[:], gi[:], ident[:])
                nc.scalar.copy(out_sb[:, blk * 128:(blk + 1) * 128], pst[:])
                # ship this block's output now so the final drain only waits
                # for the last block's small slice
                nc.sync.dma_start(out_d[:, blk * 128:(blk + 1) * 128],
                                  out_sb[:, blk * 128:(blk + 1) * 128])

            for blk in range(NBLK):
                v, p = scans(blk)
                tail(blk, v, p)

    nc.compile()
    return nc


def _build_dispatch():
    """AOT-compile the 8-core SPMD dispatch ONCE and reuse it per call.

    run_bass_kernel_spmd under axon rebuilds jax.jit(shard_map(...)) on
    every call (fresh closure -> retrace + XLA recompile + neuronx hook:
    ~160ms) and forces extra sync points (block after execute, then a
    separate shard-by-shard output fetch). Each sync point costs one
    ~80-95ms round trip through the axon tunnel, so the baseline paid
    ~3 rounds (~250-300ms/call) for a kernel whose on-device time is
    ~0.34ms. This path replicates run_bass_via_pjrt's lowering exactly
    (same _bass_exec_p custom call, same shard_map layout), but:
      - compiles once via fast_dispatch_compile (C++ fast-path dispatch,
        no per-call retrace),
      - drops the donated zero-output upload (the kernel writes every
        output element, so uninitialized result buffers are fine),
      - never blocks between dispatch and the output fetch, so upload,
        execute, and fetch pipeline into a single tunnel round trip
        (~88-92ms/call steady state; 30-call run: min 86.7 / med 90.2).
    """
    import jax
    from jax.sharding import Mesh, PartitionSpec
    try:
        from jax.experimental.shard_map import shard_map
    except ImportError:
        from jax import shard_map
    from concourse import bass2jax
    import concourse.mybir as mybir

    nc = _CACHE["nc"]
    bass2jax.install_neuronx_cc_hook()

    partition_name = (
        nc.partition_id_tensor.name if nc.partition_id_tensor else None
    )
    in_names = ["xyz", "centers"]
    out_names = ["out_idx"]
    out_avals = [jax.core.ShapedArray((KNN, K), np.uint16)]
    in_names_all = list(in_names)
    if partition_name is not None:
        in_names_all.append(partition_name)

    def _body(*args):
        operands = list(args)
        if partition_name is not None:
            operands.append(bass2jax.partition_id_tensor())
        return tuple(bass2jax._bass_exec_p.bind(
            *operands,
            out_avals=tuple(out_avals),
            in_names=tuple(in_names_all),
            out_names=tuple(out_names),
            lowering_input_output_aliases=(),
            sim_require_finite=True,
            sim_require_nnan=True,
            nc=nc,
        ))

    devices = jax.devices()[:B]
    assert len(devices) == B, f"need {B} cores, have {len(jax.devices())}"
    mesh = Mesh(np.asarray(devices), ("core",))
    shapes = [
        jax.ShapeDtypeStruct((B * N, 3), np.float32),
        jax.ShapeDtypeStruct((B * K, 3), np.float32),
    ]
    compiled = bass2jax.fast_dispatch_compile(
        lambda: jax.jit(
            shard_map(
                _body, mesh=mesh,
                in_specs=(PartitionSpec("core"),) * 2,
                out_specs=(PartitionSpec("core"),),
                check_rep=False,
            ),
            keep_unused=True,
        ).lower(*shapes).compile()
    )
    return compiled


def _kernel_slow_path(xyz: np.ndarray, centers: np.ndarray) -> np.ndarray:
    """Fallback: stock run_bass_kernel_spmd per call (correct but ~3x slower)."""
    from concourse.bass_utils import run_bass_kernel_spmd

    in_maps = [
        {
            "xyz": np.ascontiguousarray(xyz[b]).astype(np.float32, copy=False),
            "centers": np.ascontiguousarray(centers[b]).astype(np.float32, copy=False),
        }
        for b in range(B)
    ]
    res = run_bass_kernel_spmd(_CACHE["nc"], in_maps, core_ids=list(range(B)))
    return np.stack([res.results[b]["out_idx"] for b in range(B)]).astype(np.int32)


def kernel(xyz: np.ndarray, centers: np.ndarray) -> np.ndarray:
    if "nc" not in _CACHE:
        _CACHE["nc"] = _build()
    if "call" not in _CACHE:
        try:
            _CACHE["call"] = _build_dispatch()
            if _CACHE["call"] is not None:
                # Two throwaway rounds: the first 1-2 dispatches after
                # compile pay one-time client/server init (~+100ms);
                # absorb that into the build so later calls are steady.
                xw = np.zeros((B * N, 3), np.float32)
                cw = np.zeros((B * K, 3), np.float32)
                for _ in range(2):
                    (w,) = _CACHE["call"](xw, cw)
                    np.asarray(w)
        except Exception:
            _CACHE["call"] = None
    if _CACHE["call"] is None:
        return _kernel_slow_path(xyz, centers)

    xg = np.ascontiguousarray(xyz, dtype=np.float32).reshape(B * N, 3)
    cg = np.ascontiguousarray(centers, dtype=np.float32).reshape(B * K, 3)
    try:
        (out,) = _CACHE["call"](xg, cg)
        return np.asarray(out).reshape(B, KNN, K).astype(np.int32)
    except Exception:
        # transient transport/executable error: retry via the stock path
        return _kernel_slow_path(xyz, centers)


if __name__ == "__main__":
    rng = np.random.default_rng(0)
    xyz = rng.standard_normal((B, N, 3)).astype(np.float32)
    centers = rng.standard_normal((B, K, 3)).astype(np.float32)
    out = kernel(xyz=xyz, centers=centers)
    print("out", out.shape, out.dtype)



# revision 22
# speedup vs baseline: 1.0191x; 1.0191x over previous
"""KNN (retrieval_knn) Trainium2 Bass kernel, v3.

Problem: xyz (8, 16384, 3) f32, centers (8, 1024, 3) f32 ->
top-16 nearest points per center, indices, shape (8, 16, 1024) int32.

Data-parallel over batch B=8 across 8 NeuronCores (identical SPMD program).

v3 dispatch: per-call wall clock on this axon-tunneled setup is dominated
by ~80-95ms tunnel round trips, not device time (CoreSim ~0.34ms/core).
kernel() AOT-compiles the 8-core shard_map ONCE (fast_dispatch_compile)
and never blocks between dispatch and output fetch, so upload + execute +
fetch pipeline into a single round trip (~88ms/call vs ~300ms for stock
per-call run_bass_kernel_spmd). Output indices travel as u16 (N=16384
fits) to halve the return payload; host casts back to int32.

Per core:
  - Distance via -d2[k, n] = 2c.x - x_sq - c_sq computed as ONE bf16 matmul
    with a 30-row contraction: every fp32 operand is split into three bf16
    pieces (a ~= a1+a2+a3) and the significant cross products are separate
    contraction rows. HW-validated precision: ~5e-7 mean abs err (fp32-mode
    matmul: ~3e-7) at 1 cycle/col instead of fp32's 4.
  - Operand prep runs in a dense [128, 384] layout (cheap: engine cost is
    per-partition free size), then one DRAM round-trip rearranges it into
    the wide [30, 16384] moving layout.
  - ACT evicts pairs of 1024-wide PSUM tiles (2 banks each) into 2048-wide
    SBUF scan buffers; DVE max8 + max_index over those (cheaper SBUF
    access, half the scan instructions vs PSUM-direct) -> 64 candidates
    per row; merge via max8/match_replace/max8 + max_index; recover global
    indices with an exact one-hot gather; PE-transpose to [16, 128]; emit
    int32. Top-8-per-2048-tile capacity is probabilistic (top-16 members
    spread over 8 tiles): expected ~2 rows per full problem lose a rank-
    ~16 member (HW-measured: 19 total mismatches, rel err 0.008 vs the
    0.02 gate).

CoreSim: 342.6 us (baseline kernel: 630.7 us). The one-hot gather's
casts/adds/multiply and the output copy run on gpsimd/ACT (fp32 TT
mult/add + casts are Pool-legal; is_equal and free-dim reduce are not). DVE-bound (two full
max8/max_index scans = 87% of DVE busy); ACT eviction runs within ~3% of
DVE busy, so both are near-saturated. Next step would need a cheaper
first pass (bf16 2x tournament + per-partition window gather; selection
was HW-validated but all per-partition gather paths are blocked -- see
memory trn2-knn-kernel-env-facts).
"""

import numpy as np

B = 8
N = 16384
K = 1024
KNN = 16
TILE = 1024               # PSUM tile width (2 banks)
NT = N // TILE            # 16 PSUM tiles per row
SCAN = 2048               # DVE scan-tile width (SBUF, ACT-evicted)
NBLK = K // 128           # 8 center blocks
NCAND = (N // SCAN) * 8   # 64 candidates per row
NEG = -3.0e38

# 30-row split contraction layout.
# x-side rows:              w-side rows:
#  0- 8: sqx_c split l      -1
#  9-11: 1.0                -csq split l
# 12-29: per coord c (6 rows): x-side [x1,x1,x1,x2,x2,x3]
#                             w-side [v1,v2,v3,v1,v2,v1], v = 2c
NROW = 30

_CACHE = {}


def _build():
    import concourse.bass as bass
    import concourse.mybir as mybir
    import concourse.tile as tile
    from concourse import bacc
    from concourse.masks import make_identity

    fp32 = mybir.dt.float32
    bf16 = mybir.dt.bfloat16
    u16 = mybir.dt.uint16
    i32 = mybir.dt.int32

    nc = bacc.Bacc("TRN2", target_bir_lowering=False, debug=False)

    xyz_d = nc.dram_tensor("xyz", [N, 3], fp32, kind="ExternalInput")
    cen_d = nc.dram_tensor("centers", [K, 3], fp32, kind="ExternalInput")
    # u16 output (indices < 16384): halves the per-call tunnel fetch
    # payload vs int32; host casts back to int32 after the fetch.
    out_d = nc.dram_tensor("out_idx", [KNN, K], u16, kind="ExternalOutput")
    # DRAM bounce buffers for the dense->wide rearrangement
    xb_d = nc.dram_tensor("xb_bounce", [NROW * N], bf16)
    wb_d = nc.dram_tensor("wb_bounce", [NROW * K], bf16)

    PTS = N // 128            # 128 points per partition in dense layout
    CTR = K // 128            # 8 centers per partition in dense layout

    with tile.TileContext(nc) as tc:
        with (
            tc.tile_pool(name="wide", bufs=1) as wide,
            tc.tile_pool(name="prep", bufs=1) as prep,
            tc.tile_pool(name="cand", bufs=3) as cand,
            tc.tile_pool(name="small", bufs=1) as small,
            tc.tile_pool(name="ps_mm", bufs=3, space="PSUM") as ps_mm,
            tc.tile_pool(name="ps_tr", bufs=2, space="PSUM") as ps_tr,
        ):
            # ---------------- constants ----------------
            ramp = small.tile([128, NCAND], fp32, tag="ramp")
            nc.gpsimd.iota(ramp[:], [[1, NCAND]], channel_multiplier=0,
                           allow_small_or_imprecise_dtypes=True)
            offs = small.tile([128, NCAND], fp32, tag="offs")
            nc.gpsimd.iota(offs[:], [[SCAN, N // SCAN], [0, 8]],
                           channel_multiplier=0,
                           allow_small_or_imprecise_dtypes=True)
            ident = small.tile([128, 128], fp32, tag="ident")
            make_identity(nc, ident[:])

            # ---------------- dense x-side prep ----------------
            # praw[p, c*128 + j] = coord c of point 128p + j
            praw = prep.tile([128, 3 * PTS], fp32, tag="praw")
            for c in range(3):
                nc.sync.dma_start(praw[:, c * PTS:(c + 1) * PTS],
                                  xyz_d[:, c:c + 1])
            sqx = prep.tile([128, 3 * PTS], fp32, tag="sqx")
            nc.scalar.square(sqx[:], praw[:])

            # srcall[p, r*128 + j] = x-side row r of point 128p + j.
            # The split casts write DIRECTLY into srcall's column ranges
            # (with stride-0 input broadcast for the duplicated x rows),
            # removing six serial assembly copies from the startup chain.
            srcall = prep.tile([128, NROW * PTS], bf16, tag="srcall")
            # x-side views: row 12+6c+r for coord c holds [x1,x1,x1,x2,x2,x3]
            xv = srcall[:, 12 * PTS:30 * PTS].rearrange(
                "p (c r j) -> p c r j", c=3, r=6)

            # 3-way bf16 splits of praw (casts write srcall dup ranges)
            xr1 = prep.tile([128, 3 * PTS], fp32, tag="xr1")
            xr2 = prep.tile([128, 3 * PTS], fp32, tag="xr2")
            prv = praw[:].rearrange("p (c j) -> p c j", c=3)
            nc.gpsimd.tensor_copy(
                xv[:, :, 0:3, :],
                prv[:, :, None, :].broadcast_to([128, 3, 3, PTS]))
            nc.vector.tensor_tensor(
                xr1[:].rearrange("p (c j) -> p c j", c=3),
                prv, xv[:, :, 0, :], mybir.AluOpType.subtract)
            x1v = xr1[:].rearrange("p (c j) -> p c j", c=3)
            nc.gpsimd.tensor_copy(
                xv[:, :, 3:5, :],
                x1v[:, :, None, :].broadcast_to([128, 3, 2, PTS]))
            nc.vector.tensor_tensor(
                xr2[:].rearrange("p (c j) -> p c j", c=3),
                x1v, xv[:, :, 3, :], mybir.AluOpType.subtract)
            nc.gpsimd.tensor_copy(xv[:, :, 5, :],
                                  xr2[:].rearrange("p (c j) -> p c j", c=3))

            # 3-way bf16 splits of sqx -> srcall rows 0-8 directly
            qr1 = prep.tile([128, 3 * PTS], fp32, tag="qr1")
            qr2 = prep.tile([128, 3 * PTS], fp32, tag="qr2")
            nc.gpsimd.tensor_copy(srcall[:, 0 * PTS:3 * PTS], sqx[:])
            nc.vector.tensor_tensor(qr1[:], sqx[:], srcall[:, 0 * PTS:3 * PTS],
                                    mybir.AluOpType.subtract)
            nc.gpsimd.tensor_copy(srcall[:, 3 * PTS:6 * PTS], qr1[:])
            nc.vector.tensor_tensor(qr2[:], qr1[:], srcall[:, 3 * PTS:6 * PTS],
                                    mybir.AluOpType.subtract)
            nc.gpsimd.tensor_copy(srcall[:, 6 * PTS:9 * PTS], qr2[:])
            # rows 9-11: ones
            nc.gpsimd.memset(srcall[:, 9 * PTS:12 * PTS], 1.0)

            # bounce: srcall -> DRAM in (row, point) order -> wide xb
            # DRAM flat index = r*N + 128*p + j ; both APs walk (p, r, j).
            # Chunked over partition ranges so the chunked readback below
            # unblocks as early as possible.
            nc.sync.dma_start(
                xb_d.ap().rearrange("(r p j) -> p r j", r=NROW, p=128, j=PTS),
                srcall[:].rearrange("p (r j) -> p r j", r=NROW),
            )

            # ---------------- dense w-side prep ----------------
            craw = prep.tile([128, 3 * CTR], fp32, tag="craw")
            for c in range(3):
                nc.sync.dma_start(craw[:, c * CTR:(c + 1) * CTR],
                                  cen_d[:, c:c + 1])
            v2c = prep.tile([128, 3 * CTR], fp32, tag="v2c")
            nc.scalar.mul(v2c[:], craw[:], 2.0)
            sqc = prep.tile([128, 3 * CTR], fp32, tag="sqc")
            nc.scalar.square(sqc[:], craw[:])
            ncsq = prep.tile([128, CTR], fp32, tag="ncsq")
            nc.vector.tensor_tensor(ncsq[:], sqc[:, 0:CTR], sqc[:, CTR:2 * CTR],
                                    mybir.AluOpType.add)
            nc.vector.tensor_tensor(ncsq[:], ncsq[:], sqc[:, 2 * CTR:3 * CTR],
                                    mybir.AluOpType.add)
            nc.scalar.mul(ncsq[:], ncsq[:], -1.0)

            vs1 = prep.tile([128, 3 * CTR], bf16, tag="vs1")
            vr1 = prep.tile([128, 3 * CTR], fp32, tag="vr1")
            vs2 = prep.tile([128, 3 * CTR], bf16, tag="vs2")
            vr2 = prep.tile([128, 3 * CTR], fp32, tag="vr2")
            vs3 = prep.tile([128, 3 * CTR], bf16, tag="vs3")
            nc.vector.tensor_copy(vs1[:], v2c[:])
            nc.vector.tensor_tensor(vr1[:], v2c[:], vs1[:],
                                    mybir.AluOpType.subtract)
            nc.vector.tensor_copy(vs2[:], vr1[:])
            nc.vector.tensor_tensor(vr2[:], vr1[:], vs2[:],
                                    mybir.AluOpType.subtract)
            nc.vector.tensor_copy(vs3[:], vr2[:])

            cs1 = prep.tile([128, CTR], bf16, tag="cs1")
            cr1 = prep.tile([128, CTR], fp32, tag="cr1")
            cs2 = prep.tile([128, CTR], bf16, tag="cs2")
            cr2 = prep.tile([128, CTR], fp32, tag="cr2")
            cs3 = prep.tile([128, CTR], bf16, tag="cs3")
            nc.vector.tensor_copy(cs1[:], ncsq[:])
            nc.vector.tensor_tensor(cr1[:], ncsq[:], cs1[:],
                                    mybir.AluOpType.subtract)
            nc.vector.tensor_copy(cs2[:], cr1[:])
            nc.vector.tensor_tensor(cr2[:], cr1[:], cs2[:],
                                    mybir.AluOpType.subtract)
            nc.vector.tensor_copy(cs3[:], cr2[:])

            wsrc = prep.tile([128, NROW * CTR], bf16, tag="wsrc")
            # rows 0-8: -1
            nc.gpsimd.memset(wsrc[:, 0:9 * CTR], -1.0)
            # rows 9-11: -csq splits
            nc.vector.tensor_copy(wsrc[:, 9 * CTR:10 * CTR], cs1[:])
            nc.vector.tensor_copy(wsrc[:, 10 * CTR:11 * CTR], cs2[:])
            nc.vector.tensor_copy(wsrc[:, 11 * CTR:12 * CTR], cs3[:])
            # rows 12-29: per coord c: [v1, v2, v3, v1, v2, v1]
            for c in range(3):
                base = (12 + 6 * c) * CTR
                sl1 = vs1[:, c * CTR:(c + 1) * CTR]
                sl2 = vs2[:, c * CTR:(c + 1) * CTR]
                sl3 = vs3[:, c * CTR:(c + 1) * CTR]
                nc.vector.tensor_copy(wsrc[:, base + 0 * CTR:base + 1 * CTR], sl1)
                nc.vector.tensor_copy(wsrc[:, base + 1 * CTR:base + 2 * CTR], sl2)
                nc.vector.tensor_copy(wsrc[:, base + 2 * CTR:base + 3 * CTR], sl3)
                nc.vector.tensor_copy(wsrc[:, base + 3 * CTR:base + 4 * CTR], sl1)
                nc.vector.tensor_copy(wsrc[:, base + 4 * CTR:base + 5 * CTR], sl2)
                nc.vector.tensor_copy(wsrc[:, base + 5 * CTR:base + 6 * CTR], sl1)

            nc.sync.dma_start(
                wb_d.ap().rearrange("(r p j) -> p r j", r=NROW, p=128, j=CTR),
                wsrc[:].rearrange("p (r j) -> p r j", r=NROW),
            )

            # wide operands back from DRAM; xb readback in column chunks so
            # block 0's first matmuls only wait for chunk 0
            xb = wide.tile([NROW, N], bf16, tag="xb")
            xbv = xb_d.ap().rearrange("(r n) -> r n", r=NROW)
            XCH = N // 4
            for ch in range(4):
                nc.sync.dma_start(xb[:, ch * XCH:(ch + 1) * XCH],
                                  xbv[:, ch * XCH:(ch + 1) * XCH])
            wb = wide.tile([NROW, K], bf16, tag="wb")
            nc.sync.dma_start(wb[:], wb_d.ap().rearrange("(r k) -> r k", r=NROW))

            # ---------------- output accumulator ----------------
            out_sb = small.tile([KNN, K], u16, tag="out")

            # ---------------- main loop over center blocks ----------------
            # Software-pipelined: block b's merge/gather tail is emitted
            # after block b+1's scans so DVE's in-order stream never stalls
            # on the tail's Pool/ACT round trips.
            def scans(blk):
                v = cand.tile([128, NCAND], fp32, tag="v")
                p = cand.tile([128, NCAND], u16, tag="p")
                for t in range(N // SCAN):
                    # ACT evicts two PSUM tiles into one wide SBUF scan
                    # buffer: DVE's max8/max_index then pay the cheaper SBUF
                    # access penalty, half as many scan instructions, and
                    # the candidate set shrinks to 64.
                    rb = cand.tile([128, SCAN], fp32, tag="rb")
                    for g in range(SCAN // TILE):
                        ps = ps_mm.tile([128, TILE], fp32, tag="mm")
                        base = t * SCAN + g * TILE
                        for h in range(2):
                            nc.tensor.matmul(
                                ps[:, h * 512:(h + 1) * 512],
                                wb[:, blk * 128:(blk + 1) * 128],
                                xb[:, base + h * 512:base + (h + 1) * 512],
                                start=True, stop=True,
                            )
                        nc.scalar.copy(rb[:, g * TILE:(g + 1) * TILE], ps[:])
                    nc.vector.max(out=v[:, t * 8:t * 8 + 8], in_=rb[:])
                    nc.vector.max_index(p[:, t * 8:t * 8 + 8],
                                        v[:, t * 8:t * 8 + 8], rb[:])
                return v, p

            def tail(blk, v, p):

                # merge: top-16 of the 128 candidates
                w1 = cand.tile([128, 8], fp32, tag="w1")
                w2 = cand.tile([128, 8], fp32, tag="w2")
                q = cand.tile([128, KNN], u16, tag="q")
                v2 = cand.tile([128, NCAND], fp32, tag="v2")
                nc.vector.max(out=w1[:], in_=v[:])
                nc.vector.max_index(q[:, 0:8], w1[:], v[:])
                nc.vector.match_replace(out=v2[:], in_to_replace=w1[:],
                                        in_values=v[:], imm_value=NEG)
                nc.vector.max(out=w2[:], in_=v2[:])
                nc.vector.max_index(q[:, 8:16], w2[:], v2[:])

                # exact one-hot gather: GI[j] = sum_c (q[j]==c) * (p[c]+offs[c])
                # Casts/adds/the one-hot multiply run on the otherwise-idle
                # gpsimd engine (fp32 TT mult/add + casts are Pool-legal;
                # is_equal and free-dim reduce are DVE-only).
                pf = cand.tile([128, NCAND], fp32, tag="pf")
                nc.gpsimd.tensor_copy(pf[:], p[:])
                pg = cand.tile([128, NCAND], fp32, tag="pg")
                nc.gpsimd.tensor_tensor(pg[:], pf[:], offs[:],
                                        mybir.AluOpType.add)
                qf = cand.tile([128, KNN], fp32, tag="qf")
                nc.gpsimd.tensor_copy(qf[:], q[:])
                eq = cand.tile([128, KNN, NCAND], fp32, tag="eq")
                nc.vector.tensor_tensor(
                    eq[:],
                    qf[:, :, None].broadcast_to([128, KNN, NCAND]),
                    ramp[:, None, :].broadcast_to([128, KNN, NCAND]),
                    mybir.AluOpType.is_equal,
                )
                eqw = cand.tile([128, KNN, NCAND], fp32, tag="eqw")
                nc.gpsimd.tensor_tensor(
                    eqw[:], eq[:],
                    pg[:, None, :].broadcast_to([128, KNN, NCAND]),
                    mybir.AluOpType.mult,
                )
                gi = cand.tile([128, KNN], fp32, tag="gi")
                nc.vector.tensor_reduce(gi[:], eqw[:], axis=mybir.AxisListType.X,
                                        op=mybir.AluOpType.add)

                # transpose [128, 16] -> [16, 128] and emit int32
                pst = ps_tr.tile([KNN, 128], fp32, tag="tr")
                nc.tensor.transpose(pst[:], gi[:], ident[:])
                nc.scalar.copy(out_sb[:, blk * 128:(blk + 1) * 128], pst[:])
                # ship this block's output now so the final drain only waits
                # for the last block's small slice
                nc.sync.dma_start(out_d[:, blk * 128:(blk + 1) * 128],
                                  out_sb[:, blk * 128:(blk + 1) * 128])

            for blk in range(NBLK):
                v, p = scans(blk)
                tail(blk, v, p)

    nc.compile()
    return nc


def _build_dispatch():
    """AOT-compile the 8-core SPMD dispatch ONCE and reuse it per call.

    run_bass_kernel_spmd under axon rebuilds jax.jit(shard_map(...)) on
    every call (fresh closure -> retrace + XLA recompile + neuronx hook:
    ~160ms) and forces extra sync points (block after execute, then a
    separate shard-by-shard output fetch). Each sync point costs one
    ~80-95ms round trip through the axon tunnel, so the baseline paid
    ~3 rounds (~250-300ms/call) for a kernel whose on-device time is
    ~0.34ms. This path replicates run_bass_via_pjrt's lowering exactly
    (same _bass_exec_p custom call, same shard_map layout), but:
      - compiles once via fast_dispatch_compile (C++ fast-path dispatch,
        no per-call retrace),
      - drops the donated zero-output upload (the kernel writes every
        output element, so uninitialized result buffers are fine),
      - never blocks between dispatch and the output fetch, so upload,
        execute, and fetch pipeline into a single tunnel round trip
        (~88-92ms/call steady state; 30-call run: min 86.7 / med 90.2).
    """
    import jax
    from jax.sharding import Mesh, PartitionSpec
    try:
        from jax.experimental.shard_map import shard_map
    except ImportError:
        from jax import shard_map
    from concourse import bass2jax
    import concourse.mybir as mybir

    nc = _CACHE["nc"]
    bass2jax.install_neuronx_cc_hook()

    partition_name = (
        nc.partition_id_tensor.name if nc.partition_id_tensor else None
    )
    in_names = ["xyz", "centers"]
    out_names = ["out_idx"]
    out_avals = [jax.core.ShapedArray((KNN, K), np.uint16)]
    in_names_all = list(in_names)
    if partition_name is not None:
        in_names_all.append(partition_name)

    def _body(*args):
        operands = list(args)
        if partition_name is not None:
            operands.append(bass2jax.partition_id_tensor())
        return tuple(bass2jax._bass_exec_p.bind(
            *operands,
            out_avals=tuple(out_avals),
            in_names=tuple(in_names_all),
            out_names=tuple(out_names),
            lowering_input_output_aliases=(),
            sim_require_finite=True,
            sim_require_nnan=True,
            nc=nc,
        ))

    devices = jax.devices()[:B]
    assert len(devices) == B, f"need {B} cores, have {len(jax.devices())}"
    mesh = Mesh(np.asarray(devices), ("core",))
    shapes = [
        jax.ShapeDtypeStruct((B * N, 3), np.float32),
        jax.ShapeDtypeStruct((B * K, 3), np.float32),
    ]
    compiled = bass2jax.fast_dispatch_compile(
        lambda: jax.jit(
            shard_map(
                _body, mesh=mesh,
                in_specs=(PartitionSpec("core"),) * 2,
                out_specs=(PartitionSpec("core"),),
                check_rep=False,
            ),
            keep_unused=True,
        ).lower(*shapes).compile()
    )
    return compiled


def _kernel_slow_path(xyz: np.ndarray, centers: np.ndarray) -> np.ndarray:
    """Fallback: stock run_bass_kernel_spmd per call (correct but ~3x slower)."""
    from concourse.bass_utils import run_bass_kernel_spmd

    in_maps = [
        {
            "xyz": np.ascontiguousarray(xyz[b]).astype(np.float32, copy=False),
            "centers": np.ascontiguousarray(centers[b]).astype(np.float32, copy=False),
        }
        for b in range(B)
    ]
    res = run_bass_kernel_spmd(_CACHE["nc"], in_maps, core_ids=list(range(B)))
    return np.stack([res.results[b]["out_idx"] for b in range(B)]).astype(np.int32)


def kernel(xyz: np.ndarray, centers: np.ndarray) -> np.ndarray:
    if "nc" not in _CACHE:
        _CACHE["nc"] = _build()
    if "call" not in _CACHE:
        # a transient tunnel error here must not permanently demote us to
        # the ~3x-slower stock path: retry the AOT build once
        for _ in range(2):
            try:
                _CACHE["call"] = _build_dispatch()
                break
            except Exception:
                _CACHE["call"] = None
        if _CACHE["call"] is not None:
            # Two throwaway rounds: the first 1-2 dispatches after
            # compile pay one-time client/server init (~+100ms);
            # absorb that into the build so later calls are steady.
            # Warmup failure is non-fatal (the real call retries).
            try:
                xw = np.zeros((B * N, 3), np.float32)
                cw = np.zeros((B * K, 3), np.float32)
                for _ in range(2):
                    (w,) = _CACHE["call"](xw, cw)
                    np.asarray(w)
            except Exception:
                pass
    if _CACHE["call"] is None:
        return _kernel_slow_path(xyz, centers)

    xg = np.ascontiguousarray(xyz, dtype=np.float32).reshape(B * N, 3)
    cg = np.ascontiguousarray(centers, dtype=np.float32).reshape(B * K, 3)
    for _ in range(2):
        try:
            (out,) = _CACHE["call"](xg, cg)
            return np.asarray(out).reshape(B, KNN, K).astype(np.int32)
        except Exception:
            continue  # transient transport error: one fast-path retry
    # persistent failure: fall back to the stock dispatch path
    return _kernel_slow_path(xyz, centers)


if __name__ == "__main__":
    rng = np.random.default_rng(0)
    xyz = rng.standard_normal((B, N, 3)).astype(np.float32)
    centers = rng.standard_normal((B, K, 3)).astype(np.float32)
    out = kernel(xyz=xyz, centers=centers)
    print("out", out.shape, out.dtype)

